# revision 34
# baseline (speedup 1.0000x reference)
"""Causal self-attention (RoPE) Trainium2 Bass kernel, 8-core SPMD.

Sharding: core c = (batch b = c//2, head-group g = c%2). Each core computes
4 of the 8 heads for one batch element end-to-end, producing a partial
[T, C] output; the host sums the two head-group partials per batch.

Key design points (vs a straightforward fp32 kernel):

- Feature-reordered QKV projection: W_qkv rows are permuted host-side so
  the q/k projection psums come out in "A/B half-dim" layout (partition
  p = 32*head + (d%32); A tile = d 0:32, B tile = d 32:64). RoPE then
  becomes PURE ELEMENTWISE: qA = psA*cosA - psB*sinA, qB = psB*cosB +
  psA*sinB -- no rotation matmul, no PSUM-evacuation copy.

- Banded mixed precision (validated numerically, rel err ~4e-3 vs the
  2e-2 gate): attention blocks on the diagonal 512-token band use bf16
  operands (exact-causal widths); all strictly-older 128-token k-blocks
  use fp8 (e4m3) q/k/P/V with DoubleRow matmuls at 0.5 cycles/row:
  S^T via d-split DoubleRow (the two 32-d halves are the two DR groups),
  P@V via k-block-paired DoubleRow (two adjacent k-blocks per DR).
  The max error is dominated by early/low-entropy tokens, which live in
  the exact diagonal band, so fp8 on far blocks is nearly free.

- Softmax: no max-subtraction (|S| bounded, with a uniform in-exp bias
  keeping fp8 P under the e4m3 max); denominator via an extra all-ones
  column appended to V. Causal masking inside the diagonal 128-blocks is
  a gpsimd affine_select (fill 0) after exp.

- Engine balance (GpSimd cannot touch PSUM; a dma_start occupies its
  dispatching queue for the whole transfer): PE ~matmuls; ACT ~exp (the
  ~74us bottleneck) + idle-window DMAs; DVE ~PSUM evacuations, recip,
  normalize; GpSimd ~rope muls/adds, fp8 casts, masking, weight loads;
  SP ~bulk x/trig/y DMA.
"""

import numpy as np

B, T, C = 4, 2048, 512
H_TOT, HD = 8, 64
HL = 4          # heads per core
NCORES = 8

_prog_cache = {}
LAST_EXEC_NS = None
LAST_RESULTS = None


def _build_program(t=T):
    import concourse.tile as tile
    from concourse import bacc, mybir

    f32 = mybir.dt.float32
    f32r = mybir.dt.float32r
    bf16 = mybir.dt.bfloat16
    fp8 = mybir.dt.float8e4
    Exp = mybir.ActivationFunctionType.Exp
    # uniform bias inside every exp: keeps fp8 P = exp(S-b) well under the
    # e4m3 max (240) -- cancels exactly in the softmax normalization
    EXP_BIAS = -1.5
    DR = mybir.MatmulPerfMode.DoubleRow

    nt = t // 512      # 512-wide token slices
    nb = t // 128      # 128-wide token blocks

    nc = bacc.Bacc("TRN2", target_bir_lowering=False, debug=False,
                   enable_asserts=False, num_devices=NCORES)

    xT = nc.dram_tensor("xT", [C, t], f32r, kind="ExternalInput").ap()
    # trig: rows = A/B half-dim pattern, 4 kinds (cosA|cosB|sinAm|sinB)
    trigT = nc.dram_tensor("trigT", [128, 4 * t], f32r,
                           kind="ExternalInput").ap()
    wqkT = nc.dram_tensor("wqkT", [C, 512], f32r, kind="ExternalInput").ap()
    wvT = nc.dram_tensor("wvT", [C, 256], f32r, kind="ExternalInput").ap()
    woT = nc.dram_tensor("woT", [256, C], f32r, kind="ExternalInput").ap()
    ones64 = nc.dram_tensor("ones64", [1, 64], f32r, kind="ExternalInput").ap()
    y = nc.dram_tensor("y", [t, C], f32, kind="ExternalOutput").ap()

    with tile.TileContext(nc) as tc:
        with tc.tile_pool(name="persist", bufs=1) as pp, \
             tc.tile_pool(name="dscratch", bufs=4, space="DRAM") as dp, \
             tc.tile_pool(name="psum", bufs=1, space="PSUM") as ps:

            # ---- constants & weights (loads deferred to prologue) ----
            # uniform in-exp bias (see EXP_BIAS): SBUF [128,1] constant
            expb = pp.tile([128, 1], f32, tag="expb")
            on64_t = pp.tile([1, 64], f32r, tag="on64")
            wq_t = [pp.tile([128, 512], f32r, tag=f"wq{i}", name=f"wq{i}")
                    for i in range(4)]
            wv_t = [pp.tile([128, 256], f32r, tag=f"wv{i}", name=f"wv{i}")
                    for i in range(4)]
            wo_t = [pp.tile([128, 512], f32r, tag=f"wo{i}", name=f"wo{i}")
                    for i in range(2)]

            # ---- persistent activations ----
            # fp8 K (A|B half-dim split) per slice; fp8 V+ones, per-head-
            # major so adjacent k-blocks pair in one DoubleRow lhsT.
            k8_sl = [pp.tile([128, 1024], fp8, tag=f"k8_{s_}",
                             name=f"k8_{s_}") for s_ in range(nt)]
            va8 = pp.tile([128, HL * nb * 128], fp8, tag="va8")

            def load_xcos(ts, trig_eng=None, split_x=False):
                # one DMA for all 4 contraction chunks of x, one for all
                # 4 trig planes -- dispatched on the idle SP queue
                xt = pp.tile([128, 2048], f32r, tag="xs", bufs=3, name="xs")
                xv = xt.rearrange("p (c n) -> p c n", c=4)
                src_v = xT.rearrange("(c p) n -> p c n", c=4)[
                    :, :, ts * 512:(ts + 1) * 512]
                if split_x:
                    nc.sync.dma_start(out=xv[:, 0:2], in_=src_v[:, 0:2])
                    nc.gpsimd.dma_start(out=xv[:, 2:4], in_=src_v[:, 2:4])
                else:
                    nc.sync.dma_start(out=xv, in_=src_v)
                x_ts = [xt[:, cc * 512:(cc + 1) * 512] for cc in range(4)]
                tg = pp.tile([128, 2048], f32r, tag="tg", bufs=3, name="tg")
                (trig_eng or nc.sync).dma_start(
                    out=tg.rearrange("p (k n) -> p k n", k=4),
                    in_=trigT.rearrange("p (k n) -> p k n", k=4)[
                        :, :, ts * 512:(ts + 1) * 512])
                cs = {nm: tg[:, i * 512:(i + 1) * 512]
                      for i, nm in enumerate(("cA", "cB", "sA", "sB"))}
                return x_ts, cs

            def emit_qkproj(ts, isk, x_ts, cs, fast=False):
                """Project q (isk=0) or k (isk=1) for slice ts and apply
                elementwise RoPE; produce bf16 (diag) + fp8 (far) tiles."""
                fbA, fbB = 2 * isk, 2 * isk + 1
                psA = ps.tile([128, 512], f32, tag="sd", bufs=2)
                for cc in range(4):
                    nc.tensor.matmul(
                        psA[:], wq_t[cc][:, fbA * 128:(fbA + 1) * 128],
                        x_ts[cc][:], start=(cc == 0), stop=(cc == 3))
                psB = ps.tile([128, 512], f32, tag="sd", bufs=2)
                for cc in range(4):
                    nc.tensor.matmul(
                        psB[:], wq_t[cc][:, fbB * 128:(fbB + 1) * 128],
                        x_ts[cc][:], start=(cc == 0), stop=(cc == 3))
                # GpSimd cannot touch PSUM: evacuate the two psums once
                # on DVE, then all rope muls/adds run cheaply on GpSimd.
                qbf = pp.tile([128, 1024], bf16, tag=f"qbf{isk}", bufs=3,
                              name=f"qbf{isk}")
                tA1 = pp.tile([128, 512], f32, tag="tA1", bufs=3)
                tA2 = pp.tile([128, 512], f32, tag="tA2", bufs=3)
                tB1 = pp.tile([128, 512], f32, tag="tB1", bufs=3)
                tB2 = pp.tile([128, 512], f32, tag="tB2", bufs=3)
                if fast:
                    # prologue latency path: DVE reads PSUM directly
                    nc.vector.tensor_mul(out=tA1[:], in0=psA[:],
                                         in1=cs["cA"][:])
                    nc.vector.tensor_mul(out=tA2[:], in0=psB[:],
                                         in1=cs["sA"][:])
                    nc.gpsimd.tensor_add(out=qbf[:, 0:512], in0=tA1[:],
                                         in1=tA2[:])
                    nc.vector.tensor_mul(out=tB1[:], in0=psB[:],
                                         in1=cs["cB"][:])
                    nc.vector.tensor_mul(out=tB2[:], in0=psA[:],
                                         in1=cs["sB"][:])
                    nc.gpsimd.tensor_add(out=qbf[:, 512:1024], in0=tB1[:],
                                         in1=tB2[:])
                else:
                    sA_ = pp.tile([128, 512], f32, tag="evA", bufs=4)
                    sB_ = pp.tile([128, 512], f32, tag="evB", bufs=4)
                    nc.vector.tensor_copy(out=sA_[:], in_=psA[:])
                    nc.vector.tensor_copy(out=sB_[:], in_=psB[:])
                    nc.gpsimd.tensor_mul(out=tA1[:], in0=sA_[:],
                                         in1=cs["cA"][:])
                    nc.gpsimd.tensor_mul(out=tA2[:], in0=sB_[:],
                                         in1=cs["sA"][:])
                    nc.gpsimd.tensor_add(out=qbf[:, 0:512], in0=tA1[:],
                                         in1=tA2[:])
                    nc.gpsimd.tensor_mul(out=tB1[:], in0=sB_[:],
                                         in1=cs["cB"][:])
                    nc.gpsimd.tensor_mul(out=tB2[:], in0=sA_[:],
                                         in1=cs["sB"][:])
                    nc.gpsimd.tensor_add(out=qbf[:, 512:1024], in0=tB1[:],
                                         in1=tB2[:])
                if isk:
                    nc.gpsimd.tensor_copy(out=k8_sl[ts][:], in_=qbf[:])
                    return qbf
                q8 = pp.tile([128, 1024], fp8, tag="q8", bufs=3, name="q8")
                nc.gpsimd.tensor_copy(out=q8[:], in_=qbf[:])
                return qbf, q8

            def emit_vproj(ts, tbl):
                tb = ts * 4 + tbl
                x_ts = x_cur[0]
                vpsum = ps.tile([128, 256], f32, tag="sd", bufs=2)
                for cc in range(4):
                    nc.tensor.matmul(
                        vpsum[:],
                        x_ts[cc][:, tbl * 128:(tbl + 1) * 128],
                        wv_t[cc][:],
                        start=(cc == 0), stop=(cc == 3))
                # fp8 copy (far blocks), head-major at stride nb*65
                nc.vector.tensor_copy(
                    out=va8.rearrange("p (h x) -> p h x", h=HL)[
                        :, :, tb * 128:tb * 128 + 64],
                    in_=vpsum.rearrange("p (h d) -> p h d", h=HL))
                # bf16 copy (diagonal blocks)
                vb = pp.tile([128, HL * 65], bf16, tag="vabf", bufs=8,
                             name="vabf")
                nc.vector.tensor_copy(
                    out=vb.rearrange("p (h x) -> p h x", h=HL)[:, :, 0:64],
                    in_=vpsum.rearrange("p (h d) -> p h d", h=HL))
                nc.gpsimd.memset(
                    vb.rearrange("p (h x) -> p h x", h=HL)[:, :, 64:65], 1.0)
                va_bf[tb] = vb

            def emit_attn(qs, hl, pkq, q_bf, q8):
                h0 = 32 * hl
                kbf = k_bf_sl  # current slice k (bf16), for diag

                def k8v(blk):   # [32, 2, 128] d-split DR lhsT for far block
                    return k8_sl[blk // 4][h0:h0 + 32].rearrange(
                        "p (two n) -> p two n", two=2)[
                        :, :, (blk % 4) * 128:(blk % 4 + 1) * 128]

                q8v = q8[h0:h0 + 32].rearrange("p (two n) -> p two n", two=2)
                opsum = ps.tile([66, 512], f32, tag="o", bufs=2)
                # ---- far 128-blocks, fp8 DoubleRow, two blocks per pass
                for kp in range(2 * qs):
                    ka = 2 * kp
                    spsum = ps.tile([128, 1024], f32, tag="s", bufs=2)
                    nc.tensor.matmul(spsum[:, 0:512], k8v(ka), q8v[:],
                                     start=True, stop=True, perf_mode=DR,
                                     tile_position=(h0, 0))
                    nc.tensor.matmul(spsum[:, 512:1024], k8v(ka + 1), q8v[:],
                                     start=True, stop=True, perf_mode=DR,
                                     tile_position=(h0, 0))
                    pT8 = pp.tile([128, 1024], fp8, tag="pT8", bufs=4)
                    nc.scalar.activation(out=pT8[:], in_=spsum[:], func=Exp,
                                         bias=expb[:])
                    vab = hl * (nb * 128) + ka * 128
                    nc.tensor.matmul(
                        opsum[:],
                        va8[:, vab:vab + 256].rearrange(
                            "p (two m) -> p two m", two=2)[:, :, 0:66],
                        pT8.rearrange("p (two n) -> p two n", two=2),
                        start=(kp == 0), stop=False, perf_mode=DR)
                # ---- diagonal 512-band, bf16, exact causal widths
                # blocks kb cover q-cols kb*128:512 (within-slice coords)
                dA = ps.tile([128, 1024], f32, tag="s", bufs=2)
                dB = ps.tile([128, 512], f32, tag="sd", bufs=2)
                for kb in range(4):
                    w = 512 - kb * 128
                    dst, off = (dA, (0, 512)[kb]) if kb < 2 else \
                               (dB, (0, 256)[kb - 2])
                    for half in range(2):
                        nc.tensor.matmul(
                            dst[:, off:off + w],
                            kbf[h0:h0 + 32,
                                512 * half + kb * 128:512 * half + (kb + 1) * 128],
                            q_bf[h0:h0 + 32,
                                 512 * half + kb * 128:512 * half + 512],
                            start=(half == 0), stop=(half == 1),
                            tile_position=(h0, 0))
                pTa = pp.tile([128, 1024], bf16, tag="pTa", bufs=3)
                nc.scalar.activation(out=pTa[:, 0:896], in_=dA[:, 0:896],
                                     func=Exp, bias=expb[:])
                pTb = pp.tile([128, 512], bf16, tag="pTb", bufs=3)
                nc.scalar.activation(out=pTb[:, 0:384], in_=dB[:, 0:384],
                                     func=Exp, bias=expb[:])
                # zero the causal-masked triangle of each diag 128-block
                # (keep where q-col - k-row >= 0, else exactly 0.0)
                for tile_, c0 in ((pTa, 0), (pTa, 512), (pTb, 0), (pTb, 256)):
                    nc.gpsimd.affine_select(
                        out=tile_[:, c0:c0 + 128], in_=tile_[:, c0:c0 + 128],
                        compare_op=mybir.AluOpType.is_ge, fill=0.0,
                        base=0, channel_multiplier=-1, pattern=[[1, 128]])
                pslc = (pTa[:, 0:512], pTa[:, 512:896],
                        pTb[:, 0:256], pTb[:, 256:384])
                for kb in range(4):
                    nc.tensor.matmul(
                        opsum[0:65, kb * 128:512],
                        va_bf[qs * 4 + kb][:, hl * 65:(hl + 1) * 65],
                        pslc[kb],
                        start=(qs == 0 and kb == 0), stop=(kb == 3))

                # ---- normalize: recip of ones-row, broadcast, scale
                pairi, half = hl // 2, (hl % 2) * 64
                recip_t = pp.tile([1, 512], f32r, tag="recip", bufs=3)
                with nc.allow_low_precision(reason="f32 recip"):
                    nc.vector.reciprocal(out=recip_t[:], in_=opsum[64:65, :])
                if qs < nt - 1:
                    # partition-broadcast via DRAM bounce (off critical path)
                    rd = dp.tile([1, 512], f32r, tag="rd", bufs=4)
                    nc.sync.dma_start(out=rd[:], in_=recip_t[:])
                    bc = pp.tile([64, 512], f32r, tag="bc", bufs=3)
                    nc.sync.dma_start(out=bc[:],
                                      in_=rd.to_broadcast([64, 512]))
                    nc.vector.tensor_mul(out=pkq[pairi][half:half + 64, :],
                                         in0=opsum[0:64, :], in1=bc[:])
                else:
                    # final q-slice: low-latency PE broadcast (tail path)
                    ot = pp.tile([65, 512], f32r, tag="ot", bufs=2)
                    nc.vector.tensor_copy(out=ot[:], in_=opsum[0:65, :])
                    bcps = ps.tile([64, 512], f32, tag="sd", bufs=2)
                    nc.tensor.matmul(bcps[:], on64_t[:], recip_t[:],
                                     start=True, stop=True)
                    nc.vector.tensor_mul(out=pkq[pairi][half:half + 64, :],
                                         in0=ot[0:64, :], in1=bcps[:])

            ysb_cur = [None]

            def emit_outproj_tbl(qs, tbl, pkq):
                if tbl == 0:
                    ysb_cur[0] = pp.tile([128, 2048], f32, tag="ysb", bufs=2,
                                         name="ysb")
                ysb = ysb_cur[0]
                ypsum = ps.tile([128, 512], f32, tag="sd", bufs=2)
                for fc in range(2):
                    nc.tensor.matmul(
                        ypsum[:],
                        pkq[fc][:, tbl * 128:(tbl + 1) * 128],
                        wo_t[fc][:],
                        start=(fc == 0), stop=(fc == 1))
                nc.vector.tensor_copy(
                    out=ysb[:, tbl * 512:(tbl + 1) * 512], in_=ypsum[:])
                if qs == nt - 1:
                    nc.scalar.dma_start(
                        out=y[(qs * 4 + tbl) * 128:(qs * 4 + tbl + 1) * 128, :],
                        in_=ysb[:, tbl * 512:(tbl + 1) * 512])
                elif tbl == 3:
                    nc.gpsimd.dma_start(
                        out=y.rearrange("(s b p) n -> p s b n", s=nt, b=4)[
                            :, qs], in_=ysb.rearrange("p (b n) -> p b n", b=4))

            # ---- prologue: loads + full projection of slice 0 ----
            x_cur = [None]
            va_bf = [None] * nb
            x0, cs0 = load_xcos(0, trig_eng=nc.scalar, split_x=True)
            x_cur[0] = x0
            nc.gpsimd.memset(expb[:], EXP_BIAS)
            # ones columns of the fp8 V tiles (softmax denominator rows),
            # written once -- value copies never touch column 64 of a block
            nc.gpsimd.memset(
                va8.rearrange("p (h b m) -> p h b m", h=HL, b=nb)[
                    :, :, :, 64:65], 1.0)
            nc.gpsimd.memset(
                va8.rearrange("p (h b m) -> p h b m", h=HL, b=nb)[
                    :, :, :, 65:66], 0.0)
            for cc in range(4):
                nc.gpsimd.dma_start(out=wq_t[cc],
                                    in_=wqkT[cc * 128:(cc + 1) * 128, :])
            for i in range(4):
                nc.gpsimd.dma_start(out=wv_t[i],
                                    in_=wvT[i * 128:(i + 1) * 128, :])
            nc.gpsimd.dma_start(out=on64_t, in_=ones64)
            q_bf_sl, q8_cur = emit_qkproj(0, 0, x0, cs0, fast=True)
            k_bf_sl = emit_qkproj(0, 1, x0, cs0, fast=True)
            for tbl in range(4):
                emit_vproj(0, tbl)

            # ---- main loop: attention(ts) with proj(ts+1) interleaved ----
            pkq_prev = None
            for ts in range(nt):
                if ts + 1 < nt:
                    x_nxt, cs_nxt = load_xcos(ts + 1)
                else:
                    x_nxt = None
                if ts == 0:
                    for i in range(2):
                        nc.gpsimd.dma_start(out=wo_t[i],
                                            in_=woT[i * 128:(i + 1) * 128, :])
                pkq = [pp.tile([128, 512], f32r, tag=f"pkq{i}", bufs=2,
                               name=f"pkq{i}") for i in range(2)]
                nxt = {}
                last = ts == nt - 1
                ylast = [None] * 4
                for hl in range(HL):
                    emit_attn(ts, hl, pkq, q_bf_sl, q8_cur)
                    if pkq_prev is not None:
                        emit_outproj_tbl(ts - 1, hl, pkq_prev)
                    if x_nxt is not None:
                        if hl == 0:
                            nxt["q"], nxt["q8"] = emit_qkproj(
                                ts + 1, 0, x_nxt, cs_nxt)
                        elif hl == 1:
                            nxt["k"] = emit_qkproj(ts + 1, 1, x_nxt, cs_nxt)
                        elif hl == 2:
                            x_cur[0] = x_nxt
                            emit_vproj(ts + 1, 0)
                            emit_vproj(ts + 1, 1)
                        else:
                            emit_vproj(ts + 1, 2)
                            emit_vproj(ts + 1, 3)
                    if last and hl == 2:
                        # tail: heads 0/1 of pkq are final after hl=1 --
                        # run their half of the output projection under the
                        # remaining heads' attention, park it in SBUF
                        for tbl in range(4):
                            yp = ps.tile([128, 512], f32, tag="sd", bufs=2)
                            nc.tensor.matmul(
                                yp[:], pkq[0][:, tbl * 128:(tbl + 1) * 128],
                                wo_t[0][:], start=True, stop=True)
                            y0 = pp.tile([128, 512], f32, tag="ysl0", bufs=4)
                            nc.vector.tensor_copy(out=y0[:], in_=yp[:])
                            ylast[tbl] = y0
                if x_nxt is not None:
                    q_bf_sl, q8_cur = nxt["q"], nxt["q8"]
                    k_bf_sl = nxt["k"]
                pkq_prev = pkq
            for tbl in range(4):
                yp = ps.tile([128, 512], f32, tag="sd", bufs=2)
                nc.tensor.matmul(
                    yp[:], pkq_prev[1][:, tbl * 128:(tbl + 1) * 128],
                    wo_t[1][:], start=True, stop=True)
                ysb = pp.tile([128, 512], f32, tag="ysl", bufs=4)
                nc.vector.tensor_add(out=ysb[:], in0=ylast[tbl][:], in1=yp[:])
                nc.scalar.dma_start(
                    out=y[((nt - 1) * 4 + tbl) * 128:
                          ((nt - 1) * 4 + tbl + 1) * 128, :],
                    in_=ysb[:])

    nc.compile()
    return nc


def _preprocess(x, cos, sin, W_qkv, W_out, t=T):
    import ml_dtypes
    s = np.float32(np.sqrt(0.125))
    # A/B half-dim feature permutation within this core's 4 heads
    # qA col p = 32*hl + j  ->  q row (4g+hl)*64 + j        (d = j)
    # qB col p = 32*hl + j  ->  q row (4g+hl)*64 + 32 + j   (d = 32+j)
    hlv, jv = np.divmod(np.arange(128), 32)
    cosT = cos[:t].T.astype(np.float32)   # [64, t]
    sinT = sin[:t].T.astype(np.float32)
    trig_np = np.ascontiguousarray(np.concatenate(
        [cosT[jv], cosT[32 + jv], -sinT[jv], sinT[32 + jv]], 1))

    in_maps = []
    for c in range(NCORES):
        b, g = divmod(c, 2)
        rowsA = (4 * g + hlv) * 64 + jv
        rowsB = rowsA + 32
        wqk = np.concatenate([
            W_qkv[rowsA] * s, W_qkv[rowsB] * s,
            W_qkv[512 + rowsA] * s, W_qkv[512 + rowsB] * s], 0)  # [512, 512]
        wv = W_qkv[1024 + g * 256:1024 + (g + 1) * 256]
        in_maps.append({
            "xT": np.ascontiguousarray(x[b, :t].T.astype(np.float32)),
            "wqkT": np.ascontiguousarray(wqk.T.astype(np.float32)),
            "wvT": np.ascontiguousarray(wv.T.astype(np.float32)),
            "woT": np.ascontiguousarray(
                W_out.T[g * 256:(g + 1) * 256].astype(np.float32)),
            "trigT": trig_np,
            "ones64": np.ones((1, 64), np.float32),
        })
    return in_maps


def kernel(x, cos, sin, W_qkv, W_out, _trace=False):
    global LAST_EXEC_NS, LAST_RESULTS
    from concourse.bass_utils import run_bass_kernel_spmd

    x = np.asarray(x); cos = np.asarray(cos); sin = np.asarray(sin)
    W_qkv = np.asarray(W_qkv); W_out = np.asarray(W_out)

    if T not in _prog_cache:
        _prog_cache[T] = _build_program(T)
    nc = _prog_cache[T]

    in_maps = _preprocess(x, cos, sin, W_qkv, W_out)
    try:
        res = run_bass_kernel_spmd(nc, in_maps, list(range(NCORES)),
                                   trace=_trace)
    except ModuleNotFoundError:
        res = run_bass_kernel_spmd(nc, in_maps, list(range(NCORES)),
                                   trace=False)
    LAST_EXEC_NS = res.exec_time_ns
    LAST_RESULTS = res
    out = np.empty((B, T, C), np.float32)
    for b in range(B):
        out[b] = res.results[2 * b]["y"] + res.results[2 * b + 1]["y"]
    return out
